# revision 1
# baseline (speedup 1.0000x reference)
"""Sparse-attention block (local sliding-window + dilated global attention,
RoPE, gated fusion, FFN) for nn_Block_45784351375573.

Distribution: data-parallel over batch B=8 across the 8 NeuronCores
(one batch element per core; all attention windows are batch-local so
no cross-core communication is needed).

kernel(**inputs) takes FULL unsharded inputs and returns the FULL output.
"""

import numpy as np

D_MODEL = 128
N_HEADS = 8
HEAD = 16
W_LOC = 128
W_DIL = 64
EPS = 1e-5
B, T = 8, 4096


def _forward_jax(x, padding_mask, cos, sin, params):
    """Reference math (jax), used by the pmap path: runs per-device on a
    [1, T, D] shard. Mirrors reference.py exactly."""
    import jax
    import jax.numpy as jnp

    def layer_norm(x, g, b):
        m = x.mean(-1, keepdims=True)
        v = ((x - m) ** 2).mean(-1, keepdims=True)
        return (x - m) * jax.lax.rsqrt(v + EPS) * g + b

    def rotate_half(x):
        h = x.shape[-1] // 2
        return jnp.concatenate([-x[..., h:], x[..., :h]], axis=-1)

    def apply_rope(x, c, s):
        return x * c + rotate_half(x) * s

    def rotary_mha(xq, xk, xv, cos_q, sin_q, cos_k, sin_k, mask, p, pre):
        N, Lq, D = xq.shape

        def proj(x, w, b):
            return (x @ w + b).reshape(N, -1, N_HEADS, HEAD).transpose(0, 2, 1, 3)

        q = apply_rope(proj(xq, p[pre + 'wq'], p[pre + 'bq']), cos_q, sin_q)
        k = apply_rope(proj(xk, p[pre + 'wk'], p[pre + 'bk']), cos_k, sin_k)
        v = proj(xv, p[pre + 'wv'], p[pre + 'bv'])
        s = jnp.einsum('nhqd,nhkd->nhqk', q, k) / jnp.sqrt(jnp.asarray(HEAD, q.dtype))
        s = jnp.where(mask[:, None, None, :], s, jnp.asarray(-1e9, s.dtype))
        a = jax.nn.softmax(s, axis=-1)
        o = jnp.einsum('nhqk,nhkd->nhqd', a, v).transpose(0, 2, 1, 3).reshape(N, Lq, D)
        return o @ p[pre + 'wo'] + p[pre + 'bo']

    def local_attn(x, mask, cos_t, sin_t, p):
        Bb, Tt, D = x.shape
        W = W_LOC
        n = Tt // W
        m = mask[..., None].astype(x.dtype)
        x = x * m
        cos_t = cos_t * m
        sin_t = sin_t * m
        q = x.reshape(Bb * n, W, D)
        cos_q = cos_t.reshape(Bb * n, W, HEAD)[:, None]
        sin_q = sin_t.reshape(Bb * n, W, HEAD)[:, None]
        hp = W // 2

        def ctx(a):
            ap = jnp.pad(a, ((0, 0), (hp, hp), (0, 0)))
            ar = ap.reshape(Bb, n + 1, W, a.shape[-1])
            return jnp.concatenate([ar[:, :-1], ar[:, 1:]], axis=2).reshape(
                Bb * n, 2 * W, a.shape[-1])

        k = ctx(x)
        cos_k = ctx(cos_t)[:, None]
        sin_k = ctx(sin_t)[:, None]
        mp = jnp.pad(mask, ((0, 0), (hp, hp)))
        mr = mp.reshape(Bb, n + 1, W)
        km = jnp.concatenate([mr[:, :-1], mr[:, 1:]], axis=2).reshape(Bb * n, 2 * W)
        km = km.at[:, 0].set(km[:, 0] | ~km.any(-1))
        out = rotary_mha(q, k, k, cos_q, sin_q, cos_k, sin_k, km, p, 'l')
        return out.reshape(Bb, Tt, D)

    def global_attn(x, mask, cos_t, sin_t, p):
        Bb, Tt, D = x.shape
        W = W_DIL
        n = Tt // W

        def dil(a):
            return a.reshape(Bb, n, W, a.shape[-1]).transpose(0, 2, 1, 3).reshape(
                Bb * W, n, a.shape[-1])

        q = dil(x)
        cq = dil(cos_t)[:, None]
        sq = dil(sin_t)[:, None]
        km = mask.reshape(Bb, n, W).transpose(0, 2, 1).reshape(Bb * W, n)
        out = rotary_mha(q, q, q, cq, sq, cq, sq, km, p, 'g')
        return out.reshape(Bb, W, n, D).transpose(0, 2, 1, 3).reshape(Bb, Tt, D)

    Bb, Tt, D = x.shape
    cos_t = jnp.broadcast_to(cos[0, 0][None], (Bb, Tt, HEAD))
    sin_t = jnp.broadcast_to(sin[0, 0][None], (Bb, Tt, HEAD))
    p = params
    resid = x
    lo = local_attn(layer_norm(x, p['ln1_g'], p['ln1_b']), padding_mask, cos_t, sin_t, p)
    go = global_attn(layer_norm(x, p['ln2_g'], p['ln2_b']), padding_mask, cos_t, sin_t, p)
    comb = jnp.concatenate([lo, go], axis=-1)
    h = jax.nn.gelu(comb @ p['gate_w1'] + p['gate_b1'], approximate=False)
    alpha = jax.nn.sigmoid(h @ p['gate_w2'] + p['gate_b2'])
    fused = layer_norm(alpha * lo + (1 - alpha) * go, p['fln_g'], p['fln_b'])
    x = resid + fused
    resid = x
    xn = layer_norm(x, p['ln3_g'], p['ln3_b'])
    ff = jax.nn.gelu(xn @ p['ffn_w1'] + p['ffn_b1'], approximate=False) @ p['ffn_w2'] + p['ffn_b2']
    return resid + ff


_PMAP_CACHE = {}


def _kernel_pmap(x, padding_mask, cos, sin, params):
    """Data-parallel execution over the 8 NeuronCores via jax pmap:
    one batch element per core."""
    import jax
    import jax.numpy as jnp

    n_dev = min(len(jax.devices()), B)
    if 'fn' not in _PMAP_CACHE:
        _PMAP_CACHE['fn'] = jax.pmap(_forward_jax, in_axes=(0, 0, None, None, None))
    fn = _PMAP_CACHE['fn']
    # shard: [8, 1, T, D] — one batch element per device
    xs = jnp.asarray(x).reshape(n_dev, B // n_dev, T, D_MODEL)
    ms = jnp.asarray(padding_mask).reshape(n_dev, B // n_dev, T)
    out = fn(xs, ms, jnp.asarray(cos), jnp.asarray(sin),
             {k: jnp.asarray(v) for k, v in params.items()})
    return np.asarray(out).reshape(B, T, D_MODEL).astype(np.float32)


def kernel(x, padding_mask, cos, sin, params):
    try:
        from kernel_bass import kernel_bass  # noqa: F401  (same-dir optional fast path)
        return kernel_bass(x, padding_mask, cos, sin, params)
    except Exception:
        pass
    return _kernel_pmap(x, padding_mask, cos, sin, params)
